# revision 5
# baseline (speedup 1.0000x reference)
"""Trainium2 Bass kernel for the attention module:

    att_h  = h @ W_h2att.T + b_h2att             # [B, 512]
    dot    = tanh(p_att_feats + att_h[:, None])  # [B, 1024, 512]
    scores = dot @ w_alpha + b_alpha             # [B, 1024]
    weight = softmax(scores, axis=1)
    out    = einsum('bs,bsd->bd', weight, att_feats)  # [B, 2048]

Sharding: data-parallel over batch B=64 across 8 NeuronCores (8 per core).
Params tiny + replicated. b_alpha is a softmax shift -> dropped.

v4 design (~26MB/core HBM read; DMA floor ~79us):
  - att host-quantized to int8 with a per-(b,s)-row absmax/127 scale
    (halves the dominant stream; measured ~8.6e-3 rel err total). The
    scale is folded into the softmax weights (tiny [128,8] DVE mul);
    int8 tiles are upconverted to bf16 raw-valued tiles, split across
    DVE/ACT/Pool so no single engine bottlenecks.
  - p host-transposed to pT [b, h, s]. ACT computes tanh(pT + att_h[b,h])
    with att_h as a per-partition bias -> no DVE elementwise score work.
  - scores via PE matvec: sc[s-chunk part, 1] = th[h, s-chunk].T @ wa[h, 1]
    accumulated over 4 h-chunks. s-mapping: s = c*128 + q.
  - exp emits wgt [128, 8] bf16 + f32 per-partition partial sums into a
    zall column; Z-reduction + divide on host.
  - weighted sum: per-b M=1 PSUM-accumulating matmuls over [128, 2048]
    bf16 tiles; DVE copies acc->row, ACT DMAs row out.
  - DMA queues: att int8 on SP ring, pT/consts on gpsimd SWDGE,
    outputs on ACT.
"""

import numpy as np
import ml_dtypes

import concourse.bass as bass
import concourse.tile as tile
from concourse import bacc, mybir
from concourse.bass import ts
from concourse.bass_utils import run_bass_kernel_spmd

F32 = mybir.dt.float32
BF16 = mybir.dt.bfloat16
I8 = mybir.dt.int8

B_LOC = 8       # batches per core
S = 1024        # attended positions
NC_ = 8         # s-chunks (s = c*128 + q)
NP_ = 4         # att DMA c-pairs
HID = 512
NHC = 4         # h-chunks
D = 2048
DT = D // 512   # output column slices
K = 2048        # rnn_size contraction
KG = K // 128   # 16 k-groups

# engine assignment for the 64 (b, c) int8->bf16 upconvert units:
# ACT c 0-1 (early-landing, no head-of-line risk), DVE c 2-4, Pool c 5-7,
# with c=7 shifted off Pool for a few batches to balance loads.
def _conv_engine(b, c):
    if c <= 1:
        return "A"
    if c <= 4:
        return "D"
    if c == 7:
        if b == 5:
            return "D"
        if b >= 6:
            return "A"
    return "P"

_NC_CACHE = None


def build_kernel(att8_bufs=14, attb_bufs=11, pt_bufs=3, th_bufs=3):
    nc = bacc.Bacc("TRN2", target_bir_lowering=False, debug=False, num_devices=8)

    p_d = nc.dram_tensor("pT", [B_LOC, HID, S], BF16, kind="ExternalInput")
    att_d = nc.dram_tensor("att8", [B_LOC, S, D], I8, kind="ExternalInput")
    scl_d = nc.dram_tensor("scl", [128, B_LOC, NC_], F32, kind="ExternalInput")
    hT_d = nc.dram_tensor("hT", [K, B_LOC], BF16, kind="ExternalInput")
    WT_d = nc.dram_tensor("WT", [K, HID], BF16, kind="ExternalInput")
    wa_d = nc.dram_tensor("wa", [128, NHC], BF16, kind="ExternalInput")
    bh_d = nc.dram_tensor("bh", [128, NHC], F32, kind="ExternalInput")
    out_d = nc.dram_tensor("out", [B_LOC, D], F32, kind="ExternalOutput")
    z_d = nc.dram_tensor("zall", [128, B_LOC], F32, kind="ExternalOutput")

    with tile.TileContext(nc) as tc:
        with (
            tc.tile_pool(name="consts", bufs=1) as consts,
            tc.tile_pool(name="singles", bufs=1) as singles,
            tc.tile_pool(name="ptp", bufs=pt_bufs) as ptpool,
            tc.tile_pool(name="thp", bufs=th_bufs) as thpool,
            tc.tile_pool(name="wgtp", bufs=3) as wgtpool,
            tc.tile_pool(name="rowp", bufs=2) as rowpool,
            tc.tile_pool(name="att8p", bufs=att8_bufs) as att8pool,
            tc.tile_pool(name="attbp", bufs=attb_bufs) as attbpool,
            tc.tile_pool(name="ps_ah", bufs=1, space=bass.MemorySpace.PSUM) as ps_ah,
            tc.tile_pool(name="ps_sc", bufs=2, space=bass.MemorySpace.PSUM) as ps_sc,
            tc.tile_pool(name="ps_acc", bufs=5, space=bass.MemorySpace.PSUM) as ps_acc,
        ):
            att_r = [
                att_d[b].rearrange("(c q) d -> q c d", q=128) for b in range(B_LOC)
            ]
            p_r = [
                p_d[b].rearrange("(hc q) s -> q hc s", q=128) for b in range(B_LOC)
            ]

            att8_tiles = {}

            def emit_att_dma(b):
                tiles = []
                for p in range(NP_):
                    at = att8pool.tile(
                        [128, 2, D], I8, name=f"a8_{b}_{p}", tag="a8"
                    )
                    nc.sync.dma_start(at[:], att_r[b][:, 2 * p : 2 * p + 2, :])
                    tiles.append(at)
                att8_tiles[b] = tiles

            attb_tiles = {}

            def emit_convert(b):
                tiles = []
                for c in range(NC_):
                    src = att8_tiles[b][c // 2][:, c % 2, :]
                    dst = attbpool.tile([128, D], BF16, name=f"ab{b}_{c}", tag="ab")
                    eng = _conv_engine(b, c)
                    if eng == "D":
                        nc.vector.tensor_copy(dst[:], src)
                    elif eng == "A":
                        nc.scalar.copy(dst[:], src)
                    else:
                        nc.gpsimd.tensor_copy(dst[:], src)
                    tiles.append(dst)
                attb_tiles[b] = tiles

            pt_tiles = {}

            def emit_pt_dma(b):
                pt = ptpool.tile([128, NHC, S], BF16, name=f"pt{b}", tag="pt")
                nc.gpsimd.dma_start(pt[:], p_r[b])
                pt_tiles[b] = pt

            # ---- consts on gpsimd ring; att stream starts immediately on SP
            ht = consts.tile([128, KG, B_LOC], BF16)
            nc.gpsimd.dma_start(
                ht[:], hT_d.rearrange("(kg q) b -> q kg b", q=128)
            )
            wa = consts.tile([128, NHC], BF16)
            nc.gpsimd.dma_start(wa[:], wa_d[:])
            bh = consts.tile([128, NHC], F32)
            nc.gpsimd.dma_start(bh[:], bh_d[:])
            scl = consts.tile([128, B_LOC, NC_], F32)
            nc.gpsimd.dma_start(scl[:], scl_d[:])

            emit_att_dma(0)
            emit_pt_dma(0)

            wt = consts.tile([128, KG, HID], BF16)
            nc.gpsimd.dma_start(
                wt[:], WT_d.rearrange("(kg q) h -> q kg h", q=128)
            )
            emit_att_dma(1)
            emit_pt_dma(1)

            # ---- att_hT[h, b] = sum_k WT[k, h] * hT[k, b]  ([128, 4hc, 8b])
            ahT = ps_ah.tile([128, NHC * B_LOC], F32)
            for hc in range(NHC):
                for kg in range(KG):
                    nc.tensor.matmul(
                        ahT[:, ts(hc, B_LOC)],
                        wt[:, kg, ts(hc, 128)],
                        ht[:, kg, :],
                        start=(kg == 0),
                        stop=(kg == KG - 1),
                    )
            biasT = singles.tile([128, NHC, B_LOC], F32)
            for hc in range(NHC):
                nc.vector.tensor_add(
                    biasT[:, hc, :],
                    ahT[:, ts(hc, B_LOC)],
                    bh[:, hc : hc + 1].broadcast_to((128, B_LOC)),
                )

            # per-partition exp partial sums, one column per batch
            zall = singles.tile([128, B_LOC], F32)

            th_tiles = {}
            wgtT = {}

            def emit_tanh(b):
                th = thpool.tile([128, NHC, S], BF16, name=f"th{b}", tag="th")
                for hc in range(NHC):
                    nc.scalar.activation(
                        th[:, hc, :], pt_tiles[b][:, hc, :],
                        mybir.ActivationFunctionType.Tanh,
                        bias=biasT[:, hc, b : b + 1], scale=1.0,
                    )
                th_tiles[b] = th

            def emit_scores(b):
                sc = ps_sc.tile([128, NC_], F32, name=f"sc{b}", tag="sc")
                for c in range(NC_):
                    for hc in range(NHC):
                        nc.tensor.matmul(
                            sc[:, c : c + 1],
                            th_tiles[b][:, hc, ts(c, 128)],
                            wa[:, hc : hc + 1],
                            start=(hc == 0),
                            stop=(hc == NHC - 1),
                        )
                return sc

            sc_tiles = {}

            def emit_exp(b):
                # exp -> raw softmax weights; then fold in the int8 row scale
                wgt = wgtpool.tile([128, NC_], BF16, name=f"wgt{b}", tag="wgt")
                nc.scalar.activation(
                    wgt[:], sc_tiles[b][:], mybir.ActivationFunctionType.Exp,
                    accum_out=zall[:, b : b + 1],
                )
                wgs = wgtpool.tile([128, NC_], BF16, name=f"wgs{b}", tag="wgs")
                nc.vector.tensor_mul(wgs[:], wgt[:], scl[:, b, :])
                wgtT[b] = wgs

            def emit_weighted(b):
                accs = [
                    ps_acc.tile([1, 512], F32, name=f"acc{b}_{d}", tag="acc")
                    for d in range(DT)
                ]
                for c in range(NC_):
                    at = attb_tiles[b][c]
                    for d in range(DT):
                        nc.tensor.matmul(
                            accs[d][:],
                            wgtT[b][:, c : c + 1],
                            at[:, ts(d, 512)],
                            start=(c == 0),
                            stop=(c == NC_ - 1),
                        )
                row = rowpool.tile([1, D], F32, name=f"row{b}", tag="row")
                for d in range(DT):
                    nc.vector.tensor_copy(row[0:1, ts(d, 512)], accs[d][:])
                nc.scalar.dma_start(out_d[b : b + 1, :], row[:])

            # ---- prologue
            emit_tanh(0)
            sc_tiles[0] = emit_scores(0)
            emit_convert(0)
            emit_pt_dma(2)
            emit_tanh(1)
            sc_tiles[1] = emit_scores(1)
            emit_exp(0)

            for b in range(B_LOC):
                if b + 2 < B_LOC:
                    emit_att_dma(b + 2)
                emit_weighted(b)
                if b + 3 < B_LOC:
                    emit_pt_dma(b + 3)
                if b + 1 < B_LOC:
                    emit_convert(b + 1)
                if b + 2 < B_LOC:
                    emit_tanh(b + 2)
                    sc_tiles[b + 2] = emit_scores(b + 2)
                if b + 1 < B_LOC:
                    emit_exp(b + 1)

            nc.scalar.dma_start(z_d[:], zall[:])

    nc.compile()
    return nc


def _in_maps(h, att_feats, p_att_feats, W_h2att, b_h2att, w_alpha):
    bf = ml_dtypes.bfloat16
    att_f = np.asarray(att_feats, dtype=np.float32)
    amax = np.abs(att_f).max(axis=2, keepdims=True)          # [B, S, 1]
    scale = (amax / 127.0).astype(np.float32)
    att_q = np.clip(np.round(att_f / scale), -127, 127).astype(np.int8)
    # scl layout [q, b_loc, c] with s = c*128 + q
    scale_bsc = scale[:, :, 0].reshape(-1, B_LOC, NC_, 128)  # [cores, b, c, q]
    pT = np.ascontiguousarray(
        np.swapaxes(p_att_feats, 1, 2)
    ).astype(bf)                                             # [B, HID, S]
    WT = np.ascontiguousarray(W_h2att.T).astype(bf)
    wa = np.ascontiguousarray(
        w_alpha.astype(np.float32).reshape(NHC, 128).T
    ).astype(bf)
    bh = np.ascontiguousarray(
        b_h2att.astype(np.float32).reshape(NHC, 128).T
    )
    maps = []
    for c in range(8):
        sl = slice(c * B_LOC, (c + 1) * B_LOC)
        maps.append(
            {
                "pT": np.ascontiguousarray(pT[sl]),
                "att8": np.ascontiguousarray(att_q[sl]),
                "scl": np.ascontiguousarray(scale_bsc[c].transpose(2, 0, 1)),
                "hT": np.ascontiguousarray(h[sl].T.astype(bf)),
                "WT": WT,
                "wa": wa,
                "bh": bh,
            }
        )
    return maps


def kernel(h, att_feats, p_att_feats, W_h2att, b_h2att, w_alpha, b_alpha):
    global _NC_CACHE
    h = np.asarray(h)
    att_feats = np.asarray(att_feats)
    p_att_feats = np.asarray(p_att_feats)
    W_h2att = np.asarray(W_h2att)
    b_h2att = np.asarray(b_h2att)
    w_alpha = np.asarray(w_alpha)
    if _NC_CACHE is None:
        _NC_CACHE = build_kernel()
    nc = _NC_CACHE
    maps = _in_maps(h, att_feats, p_att_feats, W_h2att, b_h2att, w_alpha)
    res = run_bass_kernel_spmd(nc, maps, core_ids=list(range(8)))
    outs = []
    for c in range(8):
        row = res.results[c]["out"]                     # [8, 2048] unnormalized
        z = res.results[c]["zall"].sum(axis=0)          # [8]
        outs.append(row / z[:, None])
    return np.concatenate(outs, axis=0).astype(np.float32)


# revision 6
# speedup vs baseline: 1.3991x; 1.3991x over previous
"""Trainium2 Bass kernel for the attention module:

    att_h  = h @ W_h2att.T + b_h2att             # [B, 512]
    dot    = tanh(p_att_feats + att_h[:, None])  # [B, 1024, 512]
    scores = dot @ w_alpha + b_alpha             # [B, 1024]
    weight = softmax(scores, axis=1)
    out    = einsum('bs,bsd->bd', weight, att_feats)  # [B, 2048]

Sharding: data-parallel over batch B=64 across 8 NeuronCores (8 per core).
Params tiny + replicated. b_alpha is a softmax shift -> dropped.

v5 design (~30MB/core HBM read; DMA floor ~91us):
  - hybrid att precision: s-chunks c=0,1 stay bf16 (consumed directly);
    c=2..7 are host-quantized int8 with per-(b,s)-row absmax/127 scales
    (measured conversion rates: ACT 1.9us, DVE 4.6us, Pool 7.5us per
    [128,2048] tile -> ACT converts c2-4, DVE c5-6, Pool c7).
  - int8 row scales folded into the softmax weights, not the data.
  - weighted sum accumulates ALL batches into persistent [8, 512] PSUM
    tiles via zero-masked weight columns (lhsT [128, 8] with only column
    b nonzero) -> single tail copy + one output DMA, no per-b PSUM
    drain on DVE.
  - p host-transposed to pT [b, h, s]. ACT computes tanh(pT + att_h[b,h])
    with att_h as a per-partition bias.
  - scores via PE matvec; s-mapping: s = c*128 + q.
  - exp emits wgt [128, 8] bf16 + f32 per-partition partial sums into a
    zall column; Z-reduction + divide on host.
  - DMA queues: att on SP ring, pT/consts on gpsimd SWDGE, outputs ACT.
"""

import numpy as np
import ml_dtypes

import concourse.bass as bass
import concourse.tile as tile
from concourse import bacc, mybir
from concourse.bass import ts
from concourse.bass_utils import run_bass_kernel_spmd

F32 = mybir.dt.float32
BF16 = mybir.dt.bfloat16
I8 = mybir.dt.int8

B_LOC = 8       # batches per core
S = 1024        # attended positions
NC_ = 8         # s-chunks (s = c*128 + q)
NBF = 2         # bf16-direct s-chunks per batch (c = 0..NBF-1)
NQ = NC_ - NBF  # int8 s-chunks per batch (c = NBF..7)
HID = 512
NHC = 4         # h-chunks
D = 2048
DT = D // 512   # output column slices
K = 2048        # rnn_size contraction
KG = K // 128   # 16 k-groups

# engine for the int8->bf16 upconvert of chunk c (c = 2..7)
_CONV_ENG = {2: "A", 3: "A", 4: "A", 5: "D", 6: "D", 7: "P"}

_NC_CACHE = None


def build_kernel(abf_bufs=3, a8_bufs=9, ab_bufs=10, pt_bufs=3, th_bufs=3):
    nc = bacc.Bacc("TRN2", target_bir_lowering=False, debug=False, num_devices=8)

    p_d = nc.dram_tensor("pT", [B_LOC, HID, S], BF16, kind="ExternalInput")
    abf_d = nc.dram_tensor("attbf", [B_LOC, NBF * 128, D], BF16, kind="ExternalInput")
    a8_d = nc.dram_tensor("att8", [B_LOC, NQ * 128, D], I8, kind="ExternalInput")
    scl_d = nc.dram_tensor("scl", [128, B_LOC, NQ], F32, kind="ExternalInput")
    hT_d = nc.dram_tensor("hT", [K, B_LOC], BF16, kind="ExternalInput")
    WT_d = nc.dram_tensor("WT", [K, HID], BF16, kind="ExternalInput")
    wa_d = nc.dram_tensor("wa", [128, NHC], BF16, kind="ExternalInput")
    bh_d = nc.dram_tensor("bh", [128, NHC], F32, kind="ExternalInput")
    out_d = nc.dram_tensor("out", [B_LOC, D], F32, kind="ExternalOutput")
    z_d = nc.dram_tensor("zall", [128, B_LOC], F32, kind="ExternalOutput")

    with tile.TileContext(nc) as tc:
        with (
            tc.tile_pool(name="consts", bufs=1) as consts,
            tc.tile_pool(name="singles", bufs=1) as singles,
            tc.tile_pool(name="ptp", bufs=pt_bufs) as ptpool,
            tc.tile_pool(name="thp", bufs=th_bufs) as thpool,
            tc.tile_pool(name="wgtp", bufs=3) as wgtpool,
            tc.tile_pool(name="wgmp", bufs=3) as wgmpool,
            tc.tile_pool(name="abfp", bufs=abf_bufs) as abfpool,
            tc.tile_pool(name="a8p", bufs=a8_bufs) as a8pool,
            tc.tile_pool(name="abp", bufs=ab_bufs) as abpool,
            tc.tile_pool(name="ps_ah", bufs=1, space=bass.MemorySpace.PSUM) as ps_ah,
            tc.tile_pool(name="ps_sc", bufs=2, space=bass.MemorySpace.PSUM) as ps_sc,
            tc.tile_pool(name="ps_acc", bufs=1, space=bass.MemorySpace.PSUM) as ps_acc,
        ):
            abf_r = [
                abf_d[b].rearrange("(c q) d -> q c d", q=128) for b in range(B_LOC)
            ]
            a8_r = [
                a8_d[b].rearrange("(c q) d -> q c d", q=128) for b in range(B_LOC)
            ]
            p_r = [
                p_d[b].rearrange("(hc q) s -> q hc s", q=128) for b in range(B_LOC)
            ]

            abf_tiles = {}
            a8_tiles = {}

            def emit_att_dma(b):
                # bf16 pair c0-1, then int8 pairs (c2,3) (c4,5) (c6,7)
                at = abfpool.tile([128, NBF, D], BF16, name=f"abf{b}", tag="abf")
                nc.sync.dma_start(at[:], abf_r[b])
                abf_tiles[b] = at
                tiles = []
                for p in range(NQ // 2):
                    t = a8pool.tile([128, 2, D], I8, name=f"a8_{b}_{p}", tag="a8")
                    nc.sync.dma_start(t[:], a8_r[b][:, 2 * p : 2 * p + 2, :])
                    tiles.append(t)
                a8_tiles[b] = tiles

            ab_tiles = {}

            def emit_convert(b):
                tiles = {}
                for c in range(NBF, NC_):
                    i = c - NBF
                    src = a8_tiles[b][i // 2][:, i % 2, :]
                    dst = abpool.tile([128, D], BF16, name=f"ab{b}_{c}", tag="ab")
                    eng = _CONV_ENG[c]
                    if eng == "D":
                        nc.vector.tensor_copy(dst[:], src)
                    elif eng == "A":
                        nc.scalar.copy(dst[:], src)
                    else:
                        nc.gpsimd.tensor_copy(dst[:], src)
                    tiles[c] = dst
                ab_tiles[b] = tiles

            pt_tiles = {}

            def emit_pt_dma(b):
                pt = ptpool.tile([128, NHC, S], BF16, name=f"pt{b}", tag="pt")
                nc.gpsimd.dma_start(pt[:], p_r[b])
                pt_tiles[b] = pt

            # ---- consts on gpsimd ring; att stream starts immediately on SP
            ht = consts.tile([128, KG, B_LOC], BF16)
            nc.gpsimd.dma_start(
                ht[:], hT_d.rearrange("(kg q) b -> q kg b", q=128)
            )
            wa = consts.tile([128, NHC], BF16)
            nc.gpsimd.dma_start(wa[:], wa_d[:])
            bh = consts.tile([128, NHC], F32)
            nc.gpsimd.dma_start(bh[:], bh_d[:])
            scl = consts.tile([128, B_LOC, NQ], F32)
            nc.gpsimd.dma_start(scl[:], scl_d[:])

            emit_att_dma(0)
            emit_pt_dma(0)

            wt = consts.tile([128, KG, HID], BF16)
            nc.gpsimd.dma_start(
                wt[:], WT_d.rearrange("(kg q) h -> q kg h", q=128)
            )
            emit_att_dma(1)
            emit_pt_dma(1)

            # ---- att_hT[h, b] = sum_k WT[k, h] * hT[k, b]  ([128, 4hc, 8b])
            ahT = ps_ah.tile([128, NHC * B_LOC], F32)
            for hc in range(NHC):
                for kg in range(KG):
                    nc.tensor.matmul(
                        ahT[:, ts(hc, B_LOC)],
                        wt[:, kg, ts(hc, 128)],
                        ht[:, kg, :],
                        start=(kg == 0),
                        stop=(kg == KG - 1),
                    )
            biasT = singles.tile([128, NHC, B_LOC], F32)
            for hc in range(NHC):
                nc.vector.tensor_add(
                    biasT[:, hc, :],
                    ahT[:, ts(hc, B_LOC)],
                    bh[:, hc : hc + 1].broadcast_to((128, B_LOC)),
                )

            # per-partition exp partial sums, one column per batch
            zall = singles.tile([128, B_LOC], F32)

            # persistent weighted-sum accumulator: [8 batches, 4 dchunks, 512]
            acc = ps_acc.tile([B_LOC, DT, 512], F32)

            th_tiles = {}
            sc_tiles = {}
            wgt_tiles = {}
            wgm_tiles = {}

            def emit_tanh(b):
                th = thpool.tile([128, NHC, S], BF16, name=f"th{b}", tag="th")
                for hc in range(NHC):
                    nc.scalar.activation(
                        th[:, hc, :], pt_tiles[b][:, hc, :],
                        mybir.ActivationFunctionType.Tanh,
                        bias=biasT[:, hc, b : b + 1], scale=1.0,
                    )
                th_tiles[b] = th

            def emit_scores(b):
                sc = ps_sc.tile([128, NC_], F32, name=f"sc{b}", tag="sc")
                for c in range(NC_):
                    for hc in range(NHC):
                        nc.tensor.matmul(
                            sc[:, c : c + 1],
                            th_tiles[b][:, hc, ts(c, 128)],
                            wa[:, hc : hc + 1],
                            start=(hc == 0),
                            stop=(hc == NHC - 1),
                        )
                sc_tiles[b] = sc

            def emit_exp(b):
                wgt = wgtpool.tile([128, NC_], BF16, name=f"wgt{b}", tag="wgt")
                nc.scalar.activation(
                    wgt[:], sc_tiles[b][:], mybir.ActivationFunctionType.Exp,
                    accum_out=zall[:, b : b + 1],
                )
                wgt_tiles[b] = wgt

            def emit_wgm(b):
                # masked lhsT: [128, c, 8] zero except column b; int8 chunks
                # get the dequant row scale folded in here
                wgm = wgmpool.tile(
                    [128, NC_, B_LOC], BF16, name=f"wgm{b}", tag="wgm"
                )
                nc.vector.memset(wgm[:], 0.0)
                nc.vector.tensor_copy(
                    wgm[:, 0:NBF, b], wgt_tiles[b][:, 0:NBF]
                )
                nc.vector.tensor_mul(
                    wgm[:, NBF:NC_, b], wgt_tiles[b][:, NBF:NC_], scl[:, b, :]
                )
                wgm_tiles[b] = wgm

            def emit_weighted(b):
                for c in range(NC_):
                    if c < NBF:
                        at = abf_tiles[b][:, c, :]
                    else:
                        at = ab_tiles[b][c][:]
                    for d in range(DT):
                        nc.tensor.matmul(
                            acc[:, d, :],
                            wgm_tiles[b][:, c, :],
                            at[:, ts(d, 512)],
                            start=(b == 0 and c == 0),
                            stop=(b == B_LOC - 1 and c == NC_ - 1),
                            skip_group_check=True,
                        )

            # ---- prologue
            emit_tanh(0)
            emit_scores(0)
            emit_exp(0)
            emit_wgm(0)
            emit_convert(0)
            emit_pt_dma(2)
            emit_tanh(1)
            emit_scores(1)

            for b in range(B_LOC):
                if b + 1 < B_LOC:
                    emit_exp(b + 1)
                    emit_wgm(b + 1)
                if b + 2 < B_LOC:
                    emit_att_dma(b + 2)
                emit_weighted(b)
                if b + 3 < B_LOC:
                    emit_pt_dma(b + 3)
                if b + 1 < B_LOC:
                    emit_convert(b + 1)
                if b + 2 < B_LOC:
                    emit_tanh(b + 2)
                    emit_scores(b + 2)

            # tail: drain the persistent accumulator
            rowall = singles.tile([B_LOC, D], F32)
            for d in range(DT):
                nc.vector.tensor_copy(rowall[:, ts(d, 512)], acc[:, d, :])
            nc.scalar.dma_start(out_d[:], rowall[:])
            nc.scalar.dma_start(z_d[:], zall[:])

    nc.compile()
    return nc


def _in_maps(h, att_feats, p_att_feats, W_h2att, b_h2att, w_alpha):
    bf = ml_dtypes.bfloat16
    att_f = np.asarray(att_feats, dtype=np.float32)
    SBF = NBF * 128
    att_bf = np.ascontiguousarray(att_f[:, :SBF, :]).astype(bf)
    att_q8 = att_f[:, SBF:, :]
    amax = np.abs(att_q8).max(axis=2, keepdims=True)         # [B, 768, 1]
    scale = (amax / 127.0).astype(np.float32)
    att_q = np.clip(np.round(att_q8 / scale), -127, 127).astype(np.int8)
    # scl layout [q, b_loc, cq] with s = (cq + NBF)*128 + q
    scale_bsc = scale[:, :, 0].reshape(-1, B_LOC, NQ, 128)   # [cores, b, cq, q]
    pT = np.ascontiguousarray(
        np.swapaxes(p_att_feats, 1, 2)
    ).astype(bf)                                             # [B, HID, S]
    WT = np.ascontiguousarray(W_h2att.T).astype(bf)
    wa = np.ascontiguousarray(
        w_alpha.astype(np.float32).reshape(NHC, 128).T
    ).astype(bf)
    bh = np.ascontiguousarray(
        b_h2att.astype(np.float32).reshape(NHC, 128).T
    )
    maps = []
    for c in range(8):
        sl = slice(c * B_LOC, (c + 1) * B_LOC)
        maps.append(
            {
                "pT": np.ascontiguousarray(pT[sl]),
                "attbf": np.ascontiguousarray(att_bf[sl]),
                "att8": np.ascontiguousarray(att_q[sl]),
                "scl": np.ascontiguousarray(scale_bsc[c].transpose(2, 0, 1)),
                "hT": np.ascontiguousarray(h[sl].T.astype(bf)),
                "WT": WT,
                "wa": wa,
                "bh": bh,
            }
        )
    return maps


def kernel(h, att_feats, p_att_feats, W_h2att, b_h2att, w_alpha, b_alpha):
    global _NC_CACHE
    h = np.asarray(h)
    att_feats = np.asarray(att_feats)
    p_att_feats = np.asarray(p_att_feats)
    W_h2att = np.asarray(W_h2att)
    b_h2att = np.asarray(b_h2att)
    w_alpha = np.asarray(w_alpha)
    if _NC_CACHE is None:
        _NC_CACHE = build_kernel()
    nc = _NC_CACHE
    maps = _in_maps(h, att_feats, p_att_feats, W_h2att, b_h2att, w_alpha)
    res = run_bass_kernel_spmd(nc, maps, core_ids=list(range(8)))
    outs = []
    for c in range(8):
        row = res.results[c]["out"]                     # [8, 2048] unnormalized
        z = res.results[c]["zall"].sum(axis=0)          # [8]
        outs.append(row / z[:, None])
    return np.concatenate(outs, axis=0).astype(np.float32)
